# revision 11
# baseline (speedup 1.0000x reference)
"""Causal self-attention (B=4, T=2048, C=1024, H=16) on 8 trn2 NeuronCores.

Sharding: core c = (batch b = c//2, head-group g = c%2). Each core computes
the full attention for batch b and heads 8g..8g+7 (column-parallel qkv,
row-parallel proj), producing a partial [T, C] output; the host sums the two
partials per batch.

Per-core device kernel (Bass/Tile, SPMD same program on all 8 cores):
  qT/kT  [512, T] = (wq|wk).T @ x.T        (bf16 matmuls, fp32 psum)
  v      [T, 8, 65]  (natural layout, ones column appended per head)
  S^T    [tk 128, tq 512] blocks = kT.T-slices @ qT-slices (2 heads
         row-packed; the K=64 pair runs concurrently via PE row tiling)
  P^T    = exp((S^T + causal_mask)/8)      (ScalarE, psum->sbuf bf16)
  y/l    = [v|1].T @ P^T  accumulated over tk  -> [65, tq] psum per head
  yT_n   = yT * (1/l)   (replicate matmul broadcasts l, recip+mul on DVE)
  out    = yT_n.T @ wo  -> [T, C] fp32 partial

Scheduling: the attention inner loop is latency-bound on the
S->mask->exp->AV chain, so AV flushes run two blocks behind S, and all
non-attention matmuls (qkv projections, v tiles, output proj) are braided
INTO the attention block loop as "filler" steps drawn from a global queue,
front-loaded so each chunk's fillers finish before its last block.  Filler
units alternate between two single-bank psum pools so a unit's psum->sbuf
CAST has a full unit of PE work to drain before its bank is reused.  Input
DMAs are split across the two hardware DGE queues (sync + scalar) ordered
so the k-major v/qk startup wave can begin as soon as wv + x chunk 0 land.
"""

import os
import sys

import numpy as np

import concourse.bacc as bacc
import concourse.bass as bass
import concourse.mybir as mybir
import concourse.tile as tile
from concourse import library_config
from concourse.bass_utils import run_bass_kernel_spmd

try:
    import ml_dtypes

    BF16 = np.dtype(ml_dtypes.bfloat16)
except ImportError:  # pragma: no cover
    BF16 = np.dtype("bfloat16")

B, T, C = 4, 2048, 1024
N_HEAD = 16
D = 64  # head dim
H_LOC = 8  # heads per core
DL = H_LOC * D  # 512, local d width per core
CK = C // 128  # 8 contraction chunks
DT = mybir.dt.bfloat16
F32 = mybir.dt.float32
NEG = -1.0e9


def build_program(t_len=T, enable_asserts=False):
    """Build the SPMD per-core program. Returns the compiled Bacc object."""
    NJ = t_len // 512  # tq chunks
    NTT = t_len // 128  # 128-wide t tiles
    MD = DL // 128  # 4 d-chunks of qT/kT/yT

    nc = bacc.Bacc(
        "TRN2",
        target_bir_lowering=False,
        debug=False,
        enable_asserts=enable_asserts,
        num_devices=8,
    )

    xT_d = nc.dram_tensor("xT", [C, t_len], DT, kind="ExternalInput").ap()
    wq_d = nc.dram_tensor("wq", [C, DL], DT, kind="ExternalInput").ap()
    wk_d = nc.dram_tensor("wk", [C, DL], DT, kind="ExternalInput").ap()
    wv_d = nc.dram_tensor("wv", [C, DL], DT, kind="ExternalInput").ap()
    wo_d = nc.dram_tensor("wo", [DL, C], DT, kind="ExternalInput").ap()
    mask_d = nc.dram_tensor("mask", [128, 128], F32, kind="ExternalInput").ap()
    out_d = nc.dram_tensor("out", [t_len, C], F32, kind="ExternalOutput").ap()

    with tile.TileContext(nc) as tc:
        with (
            tc.tile_pool(name="consts", bufs=1) as cpool,
            tc.tile_pool(name="ptp", bufs=6) as pt_pool,
            tc.tile_pool(name="yup", bufs=3) as yu_pool,
            tc.tile_pool(name="lbp", bufs=3) as lb_pool,
            tc.tile_pool(name="rlp", bufs=3) as rl_pool,
            tc.tile_pool(name="outp", bufs=3) as out_pool,
            tc.tile_pool(name="psum", bufs=1, space="PSUM") as psum,
        ):
            # ---- persistent SBUF tensors ----
            xt_t = cpool.tile([128, CK, t_len], DT, name="xt")
            wq_t = cpool.tile([128, CK, DL], DT, name="wqt")
            wk_t = cpool.tile([128, CK, DL], DT, name="wkt")
            wv_t = cpool.tile([128, CK, DL], DT, name="wvt")
            wo_t = cpool.tile([128, MD, C], DT, name="wot")
            qt_t = cpool.tile([128, MD, t_len], DT, name="qtt")
            kt_t = cpool.tile([128, MD, t_len], DT, name="ktt")
            v_t = cpool.tile([128, NTT, H_LOC, D + 1], DT, name="vt")
            yt_t = cpool.tile([128, MD, t_len], DT, name="ytt")
            mask_t = cpool.tile([128, 2, 128], F32, name="maskt")
            ones1_t = cpool.tile([D + 1, 64], DT, name="ones1t")

            # ---- input DMAs split across the two HWDGE queues ----
            xT_v = xT_d.rearrange("(k p) t -> p k t", p=128)
            wq_v = wq_d.rearrange("(k p) d -> p k d", p=128)
            wk_v = wk_d.rearrange("(k p) d -> p k d", p=128)
            wv_v = wv_d.rearrange("(k p) d -> p k d", p=128)
            wo_v = wo_d.rearrange("(m p) c -> p m c", p=128)
            # x is split by (k-chunk, T-half): the first half of every
            # chunk (t 0-1023) covers v tiles 0-7 and tq chunks 0-1, so
            # attention can start after ~3MB instead of the full 5MB.
            # Chunks alternate between the sync and scalar HWDGE queues.
            nc.sync.dma_start(out=wv_t[:, 0:4, :], in_=wv_v[:, 0:4, :])
            nc.scalar.dma_start(out=wv_t[:, 4:CK, :], in_=wv_v[:, 4:CK, :])
            HT = t_len // 2
            for k in range(0, CK, 2):
                nc.sync.dma_start(out=xt_t[:, k, 0:HT], in_=xT_v[:, k, 0:HT])
                nc.scalar.dma_start(
                    out=xt_t[:, k + 1, 0:HT], in_=xT_v[:, k + 1, 0:HT]
                )
                if k == 0:
                    nc.sync.dma_start(
                        out=wq_t[:, :, 0:128], in_=wq_v[:, :, 0:128]
                    )
                    nc.scalar.dma_start(
                        out=wk_t[:, :, 0:128], in_=wk_v[:, :, 0:128]
                    )
                    nc.scalar.dma_start(out=mask_t[:, 0, :], in_=mask_d)
                    nc.scalar.dma_start(out=mask_t[:, 1, :], in_=mask_d)
            for k in range(0, CK, 2):
                nc.sync.dma_start(out=xt_t[:, k, HT:], in_=xT_v[:, k, HT:])
                nc.scalar.dma_start(
                    out=xt_t[:, k + 1, HT:], in_=xT_v[:, k + 1, HT:]
                )
            nc.sync.dma_start(out=wq_t[:, :, 128:DL], in_=wq_v[:, :, 128:DL])
            nc.scalar.dma_start(out=wk_t[:, :, 128:DL], in_=wk_v[:, :, 128:DL])
            nc.sync.dma_start(out=wo_t[:, :, :], in_=wo_v)
            # ones column (index 0) for the l (softmax denominator) rows
            nc.vector.memset(v_t[:, :, :, D : D + 1], 1.0)
            nc.vector.memset(ones1_t[:, :], 1.0)

            # ---- filler infrastructure ----
            # units alternate between the two single-bank psum names so a
            # unit's CAST has a full unit of PE work before its bank reuse
            pool_names = ["qkvps", "fillps"]
            pool_idx = [0]

            def next_ps():
                name = pool_names[pool_idx[0] % 2]
                pool_idx[0] += 1
                return psum.tile([128, 512], F32, name=name, bufs=1)

            def qk_unit(w_t, dst_t, m, j):
                """q/k projection for (m, j): 8 accumulating MMs + CAST."""
                ps = next_ps()
                for k in range(CK):
                    yield lambda k=k, ps=ps: nc.tensor.matmul(
                        ps[:, :],
                        lhsT=w_t[:, k, 128 * m : 128 * (m + 1)],
                        rhs=xt_t[:, k, 512 * j : 512 * (j + 1)],
                        start=(k == 0),
                        stop=(k == CK - 1),
                    )
                yield lambda ps=ps: nc.vector.tensor_copy(
                    dst_t[:, m, 512 * j : 512 * (j + 1)], ps[:, :]
                )

            def v_unit(ti):
                """v projection for t-tile ti: 8 accumulating MMs + CAST."""
                ps = next_ps()
                for k in range(CK):
                    yield lambda k=k, ps=ps: nc.tensor.matmul(
                        ps[:, :],
                        lhsT=xt_t[:, k, 128 * ti : 128 * (ti + 1)],
                        rhs=wv_t[:, k, :],
                        start=(k == 0),
                        stop=(k == CK - 1),
                    )
                yield lambda ps=ps: nc.vector.tensor_copy(
                    v_t[:, ti, :, 0:D],
                    ps[:, :].rearrange("p (h d) -> p h d", h=H_LOC),
                )

            out_tiles = {}

            def proj_unit(ti, ci):
                """proj half-tile (ti, ci): 4 accumulating MMs + COPY + DMA."""
                tt = slice(128 * ti, 128 * (ti + 1))
                cs = slice(512 * ci, 512 * (ci + 1))
                ot = out_tiles[ti]
                ps = next_ps()
                for hp2 in range(MD):
                    yield lambda hp2=hp2, ps=ps: nc.tensor.matmul(
                        ps[:, :],
                        lhsT=yt_t[:, hp2, tt],
                        rhs=wo_t[:, hp2, cs],
                        start=(hp2 == 0),
                        stop=(hp2 == MD - 1),
                    )

                def fin(ps=ps):
                    nc.vector.tensor_copy(ot[:, cs], ps[:, :])
                    nc.sync.dma_start(out=out_d[tt, cs], in_=ot[:, cs])

                yield fin

            def proj_chunk_units(j):
                for ti in range(4 * j, 4 * (j + 1)):
                    out_tiles[ti] = out_pool.tile([128, C], F32, name="ot")
                    yield from proj_unit(ti, 0)
                    yield from proj_unit(ti, 1)

            fillers = []

            def add_unit(gen):
                fillers.extend(gen)

            def drain_fillers(n):
                for _ in range(min(n, len(fillers))):
                    fillers.pop(0)()

            def drain_all_fillers():
                drain_fillers(len(fillers))

            # ---- startup wave: v tiles 0-3 + qk(0,0), k-major so each
            # k-step consumes x chunk k as it streams in
            def wave0():
                spsT = psum.tile([128, 2, 512], F32, name="sps", bufs=2)
                accT = psum.tile([128, 2, 512], F32, name="acc", bufs=1)
                vps = [
                    psum.tile([128, 512], F32, name="qkvps", bufs=1),
                    psum.tile([128, 512], F32, name="fillps", bufs=1),
                    spsT[:, 0, :],
                    spsT[:, 1, :],
                ]
                qps, kps = accT[:, 0, :], accT[:, 1, :]
                for k in range(CK):
                    for idx, ti in enumerate([0, 1, 2, 3]):
                        nc.tensor.matmul(
                            vps[idx],
                            lhsT=xt_t[:, k, 128 * ti : 128 * (ti + 1)],
                            rhs=wv_t[:, k, :],
                            start=(k == 0),
                            stop=(k == CK - 1),
                        )
                    nc.tensor.matmul(
                        qps,
                        lhsT=wq_t[:, k, 0:128],
                        rhs=xt_t[:, k, 0:512],
                        start=(k == 0),
                        stop=(k == CK - 1),
                    )
                    nc.tensor.matmul(
                        kps,
                        lhsT=wk_t[:, k, 0:128],
                        rhs=xt_t[:, k, 0:512],
                        start=(k == 0),
                        stop=(k == CK - 1),
                    )
                for idx, ti in enumerate([0, 1, 2, 3]):
                    nc.vector.tensor_copy(
                        v_t[:, ti, :, 0:D],
                        vps[idx].rearrange("p (h d) -> p h d", h=H_LOC),
                    )
                nc.vector.tensor_copy(qt_t[:, 0, 0:512], qps)
                nc.vector.tensor_copy(kt_t[:, 0, 0:512], kps)

            # ---- attention chunk with braided fillers ----
            # blocks are processed in PAIRS (4 S matmuls, fillers, then 4
            # AV matmuls of the pair two back) to halve PE shape
            # transitions; fillers drain BEFORE the S group so the in-order
            # PE queue is not head-of-line blocked by a waiting S.
            def attn_j(hp, j, quota=0, pe_fin=False):
                tq0 = 512 * j
                nblk = 4 * j + 4  # causal: tk blocks 0 .. 4j+3
                acc = psum.tile([128, 2, 512], F32, name="acc", bufs=1)
                pend = []  # per-block AV pairs, flushed one pair behind
                emitted = [0]
                npair = nblk // 2

                def flush_pair():
                    for blk in pend[:2]:
                        for mm in blk:
                            nc.tensor.matmul(**mm)
                    del pend[:2]

                def emit_block(i):
                    tk = slice(128 * i, 128 * (i + 1))
                    diag = i - 4 * j
                    lo = 128 * diag if diag >= 0 else 0
                    tqs = slice(tq0 + lo, tq0 + 512)
                    sps = psum.tile([128, 2, 512], F32, name="sps", bufs=2)
                    for h2, lohi in ((0, slice(0, 64)), (1, slice(64, 128))):
                        nc.tensor.matmul(
                            sps[:, h2, lo:],
                            lhsT=kt_t[lohi, hp, tk],
                            rhs=qt_t[lohi, hp, tqs],
                            start=True,
                            stop=True,
                        )
                    if diag >= 0:  # block crosses the causal diagonal
                        dg = slice(lo, lo + 128)
                        nc.vector.tensor_add(
                            sps[:, :, dg], sps[:, :, dg], mask_t[:, :, :]
                        )
                    pt = pt_pool.tile([128, 2, 512], DT, name="pt")
                    nc.scalar.activation(
                        pt[:, :, lo:],
                        sps[:, :, lo:],
                        mybir.ActivationFunctionType.Exp,
                        scale=0.125,
                    )
                    pend.append(
                        [
                            dict(
                                out=acc[0 : D + 1, h2, lo:],
                                lhsT=v_t[:, i, 2 * hp + h2, :],
                                rhs=pt[:, h2, lo:],
                                start=(i == 0),
                                stop=(i == nblk - 1),
                            )
                            for h2 in range(2)
                        ]
                    )

                for p in range(npair):
                    want = min(quota, (quota * (p + 1)) // max(npair - 1, 1))
                    drain_fillers(want - emitted[0])
                    emitted[0] = want
                    emit_block(2 * p)
                    emit_block(2 * p + 1)
                    if len(pend) >= 4:
                        flush_pair()
                while pend:
                    flush_pair()

                # normalization: copy l/y rows to SBUF now (frees acc); the
                # rest is deferred.  pe_fin (last chunk): replicate-matmul
                # broadcast on the PE for a shorter serial tail chain.
                tq = slice(tq0, tq0 + 512)
                yu = yu_pool.tile([D + 1, 2, 512], DT, name="yu")
                nc.vector.tensor_copy(yu[:, 0, :], acc[0 : D + 1, 0, :])
                nc.vector.tensor_copy(yu[:, 1, :], acc[0 : D + 1, 1, :])

                def finish_pe():
                    for h2 in range(2):
                        nc.tensor.matmul(
                            acc[0:64, h2, :],
                            lhsT=ones1_t[D : D + 1, :],
                            rhs=yu[D : D + 1, h2, :],
                            start=True,
                            stop=True,
                            tile_position=(64, 0),
                        )
                    rl = rl_pool.tile([64, 2, 512], F32, name="rl")
                    nc.vector.reciprocal_approx_fast(rl[:, 0, :], acc[0:64, 0, :])
                    nc.vector.reciprocal_approx_fast(rl[:, 1, :], acc[0:64, 1, :])
                    nc.vector.tensor_mul(
                        yt_t[0:64, hp, tq], yu[0:D, 0, :], rl[:, 0, :]
                    )
                    nc.vector.tensor_mul(
                        yt_t[64:128, hp, tq], yu[0:D, 1, :], rl[:, 1, :]
                    )

                return finish_pe

            # ---- schedule ----
            pend_fin = [None]

            def run_fin():
                if pend_fin[0] is not None:
                    pend_fin[0]()
                    pend_fin[0] = None

            wave0()

            # stage m=0: braid v tiles and next qk chunk into attention
            for j in range(NJ):
                if j < NJ - 1:
                    for ti in range(4 * (j + 1), 4 * (j + 2)):
                        add_unit(v_unit(ti))
                    add_unit(qk_unit(wq_t, qt_t, 0, j + 1))
                    add_unit(qk_unit(wk_t, kt_t, 0, j + 1))
                else:
                    add_unit(qk_unit(wq_t, qt_t, 1, 0))
                    add_unit(qk_unit(wk_t, kt_t, 1, 0))
                run_fin()
                quota = len(fillers)
                pend_fin[0] = attn_j(0, j, quota)
                drain_all_fillers()

            # stages m=1,2: braid the next chunk's qk
            for m in range(1, MD - 1):
                for j in range(NJ):
                    if j < NJ - 1:
                        add_unit(qk_unit(wq_t, qt_t, m, j + 1))
                        add_unit(qk_unit(wk_t, kt_t, m, j + 1))
                    else:
                        add_unit(qk_unit(wq_t, qt_t, m + 1, 0))
                        add_unit(qk_unit(wk_t, kt_t, m + 1, 0))
                    run_fin()
                    quota = len(fillers)
                    pend_fin[0] = attn_j(m, j, quota)
                    drain_all_fillers()

            # stage m=3: braid next qk + the previous chunk's output proj
            m = MD - 1
            for j in range(NJ):
                if j < NJ - 1:
                    add_unit(qk_unit(wq_t, qt_t, m, j + 1))
                    add_unit(qk_unit(wk_t, kt_t, m, j + 1))
                run_fin()  # finish of chunk j-1 -> yt(j-1) complete
                if j > 0:
                    add_unit(proj_chunk_units(j - 1))
                quota = len(fillers)
                pend_fin[0] = attn_j(m, j, quota, pe_fin=(j == NJ - 1))
                drain_all_fillers()
            # tail: finish last chunk (PE replicate variant), then its proj
            run_fin()
            for step in proj_chunk_units(NJ - 1):
                step()

    nc.compile()
    return nc


def make_host_inputs(x, w_qkv, w_proj, t_len=T):
    """Shard full inputs into the 8 per-core input dicts."""
    mask = np.where(
        np.arange(128)[None, :] >= np.arange(128)[:, None], 0.0, NEG
    ).astype(np.float32)

    in_maps = []
    for c in range(8):
        b, g = c // 2, c % 2
        xT = np.ascontiguousarray(x[b][:t_len].T).astype(BF16)
        wq = w_qkv[:, 512 * g : 512 * (g + 1)].astype(BF16)
        wk = w_qkv[:, C + 512 * g : C + 512 * (g + 1)].astype(BF16)
        wv = w_qkv[:, 2 * C + 512 * g : 2 * C + 512 * (g + 1)].astype(BF16)
        wo = np.ascontiguousarray(w_proj[512 * g : 512 * (g + 1), :]).astype(BF16)
        in_maps.append(dict(xT=xT, wq=wq, wk=wk, wv=wv, wo=wo, mask=mask))
    return in_maps


_CACHE = {}


def _get_program():
    if "nc" not in _CACHE:
        _CACHE["nc"] = build_program()
    return _CACHE["nc"]


def kernel(x, w_qkv, w_proj, _trace=False, _trace_kwargs=None):
    x = np.asarray(x, np.float32)
    w_qkv = np.asarray(w_qkv, np.float32)
    w_proj = np.asarray(w_proj, np.float32)
    nc = _get_program()
    in_maps = make_host_inputs(x, w_qkv, w_proj)
    kw = {}
    if _trace:
        kw = dict(trace=True, **(_trace_kwargs or {}))
    res = run_bass_kernel_spmd(nc, in_maps, core_ids=list(range(8)), **kw)
    out = np.empty((B, T, C), np.float32)
    for b in range(B):
        out[b] = res.results[2 * b]["out"] + res.results[2 * b + 1]["out"]
    if _trace:
        return out, res
    return out


# revision 13
# speedup vs baseline: 1.0730x; 1.0730x over previous
"""Causal self-attention (B=4, T=2048, C=1024, H=16) on 8 trn2 NeuronCores.

Sharding: core c = (batch b = c//2, head-group g = c%2). Each core computes
the full attention for batch b and heads 8g..8g+7 (column-parallel qkv,
row-parallel proj), producing a partial [T, C] output; the host sums the two
partials per batch.

Per-core device kernel (Bass/Tile, SPMD same program on all 8 cores):
  qT/kT  [512, T] = (wq|wk).T @ x.T        (bf16 matmuls, fp32 psum)
  v      [T, 8, 65]  (natural layout, ones column appended per head)
  S^T    [tk 128, tq 512] blocks = kT.T-slices @ qT-slices (2 heads
         row-packed; the K=64 pair runs concurrently via PE row tiling)
  P^T    = exp((S^T + causal_mask)/8)      (ScalarE, psum->sbuf bf16)
  y/l    = [v|1].T @ P^T  accumulated over tk  -> [65, tq] psum per head
  yT_n   = yT * (1/l)   (l row moved to partition 0 by an SBUF->SBUF DMA,
         broadcast across partitions on GpSimd, recip+mul on DVE; the last
         chunk uses a PE replicate matmul for a shorter serial tail)
  out    = yT_n.T @ wo  -> [T, C] fp32 partial

Scheduling: the Tile scheduler is dependency+priority driven, so emission
order is a priority hint.  All non-attention matmuls (qkv projections, v
tiles, output proj) are braided INTO the attention block loop as "filler"
steps, front-loaded per chunk; filler units alternate between two
single-bank psum pools.  wq/wk are sent by the host in m-major layout so
each 128-column slice is a contiguous fast DMA; x is split by
(k-chunk, T-half) across the two HWDGE queues so the v/qk startup wave
starts as soon as wv + x chunk 0 land and attention by ~19us.
"""

import os
import sys

import numpy as np

import concourse.bacc as bacc
import concourse.bass as bass
import concourse.bass_utils as _bass_utils
import concourse.mybir as mybir
import concourse.tile as tile
from concourse import library_config
from concourse.bass_utils import run_bass_kernel_spmd

try:
    import ml_dtypes

    BF16 = np.dtype(ml_dtypes.bfloat16)
except ImportError:  # pragma: no cover
    BF16 = np.dtype("bfloat16")

# walrus's LDWEIGHTS optimization rejects tile_position'd ldweights
# ("InstLdweights is not compatible with LDW optimization"), so it stays off.
ENABLE_LDW_OPT = False
if ENABLE_LDW_OPT and not getattr(_bass_utils, "_ldw_patch", False):
    _orig_run_command = _bass_utils.run_command

    def _run_command_ldw(argv, **kwargs):
        argv = [
            "--enable-ldw-opt=true" if a == "--enable-ldw-opt=false" else a
            for a in argv
        ]
        return _orig_run_command(argv, **kwargs)

    _bass_utils.run_command = _run_command_ldw
    _bass_utils._ldw_patch = True

B, T, C = 4, 2048, 1024
N_HEAD = 16
D = 64  # head dim
H_LOC = 8  # heads per core
DL = H_LOC * D  # 512, local d width per core
CK = C // 128  # 8 contraction chunks
MD = DL // 128  # 4 d-chunks of qT/kT/yT
DT = mybir.dt.bfloat16
F32 = mybir.dt.float32
NEG = -1.0e9


def build_program(t_len=T, enable_asserts=False):
    """Build the SPMD per-core program. Returns the compiled Bacc object."""
    NJ = t_len // 512  # tq chunks
    NTT = t_len // 128  # 128-wide t tiles

    nc = bacc.Bacc(
        "TRN2",
        target_bir_lowering=False,
        debug=False,
        enable_asserts=enable_asserts,
        num_devices=8,
    )

    xT_d = nc.dram_tensor("xT", [C, t_len], DT, kind="ExternalInput").ap()
    # wq/wk arrive m-major: [m, p, k, 128] so each m-slice DMA is contiguous
    wq_d = nc.dram_tensor("wq", [MD, 128, CK, 128], DT, kind="ExternalInput").ap()
    wk_d = nc.dram_tensor("wk", [MD, 128, CK, 128], DT, kind="ExternalInput").ap()
    wv_d = nc.dram_tensor("wv", [C, DL], DT, kind="ExternalInput").ap()
    wo_d = nc.dram_tensor("wo", [DL, C], DT, kind="ExternalInput").ap()
    mask_d = nc.dram_tensor("mask", [128, 128], F32, kind="ExternalInput").ap()
    out_d = nc.dram_tensor("out", [t_len, C], F32, kind="ExternalOutput").ap()

    with tile.TileContext(nc) as tc:
        with (
            tc.tile_pool(name="consts", bufs=1) as cpool,
            tc.tile_pool(name="ptp", bufs=6) as pt_pool,
            tc.tile_pool(name="yup", bufs=3) as yu_pool,
            tc.tile_pool(name="lrp", bufs=3) as lr_pool,
            tc.tile_pool(name="lbp", bufs=3) as lb_pool,
            tc.tile_pool(name="rlp", bufs=3) as rl_pool,
            tc.tile_pool(name="outp", bufs=3) as out_pool,
            tc.tile_pool(name="psum", bufs=1, space="PSUM") as psum,
        ):
            # ---- persistent SBUF tensors ----
            xt_t = cpool.tile([128, CK, t_len], DT, name="xt")
            wq_t = cpool.tile([128, MD, CK, 128], DT, name="wqt")
            wk_t = cpool.tile([128, MD, CK, 128], DT, name="wkt")
            wv_t = cpool.tile([128, CK, DL], DT, name="wvt")
            wo_t = cpool.tile([128, MD, C], DT, name="wot")
            qt_t = cpool.tile([128, MD, t_len], DT, name="qtt")
            kt_t = cpool.tile([128, MD, t_len], DT, name="ktt")
            v_t = cpool.tile([128, NTT, H_LOC, D + 1], DT, name="vt")
            yt_t = cpool.tile([128, MD, t_len], DT, name="ytt")
            mask_t = cpool.tile([128, 2, 128], F32, name="maskt")
            ones1_t = cpool.tile([D + 1, 64], DT, name="ones1t")

            # gpsimd library for partition_broadcast (normalization)
            nc.gpsimd.load_library(library_config.attn)

            # ---- input DMAs: x split by (k-chunk, T-half) alternating the
            # two HWDGE queues; wq/wk m-slices are contiguous (host m-major)
            xT_v = xT_d.rearrange("(k p) t -> p k t", p=128)
            wq_v = wq_d.rearrange("m p k d -> p m k d")
            wk_v = wk_d.rearrange("m p k d -> p m k d")
            wv_v = wv_d.rearrange("(k p) d -> p k d", p=128)
            wo_v = wo_d.rearrange("(m p) c -> p m c", p=128)
            HT = t_len // 2
            nc.sync.dma_start(out=wv_t[:, 0:4, :], in_=wv_v[:, 0:4, :])
            nc.scalar.dma_start(out=wv_t[:, 4:CK, :], in_=wv_v[:, 4:CK, :])
            for k in range(0, CK, 2):
                nc.sync.dma_start(out=xt_t[:, k, 0:HT], in_=xT_v[:, k, 0:HT])
                nc.scalar.dma_start(
                    out=xt_t[:, k + 1, 0:HT], in_=xT_v[:, k + 1, 0:HT]
                )
                if k == 0:
                    nc.sync.dma_start(out=wq_t[:, 0, :, :], in_=wq_v[:, 0, :, :])
                    nc.scalar.dma_start(
                        out=wk_t[:, 0, :, :], in_=wk_v[:, 0, :, :]
                    )
                    nc.scalar.dma_start(out=mask_t[:, 0, :], in_=mask_d)
                    nc.scalar.dma_start(out=mask_t[:, 1, :], in_=mask_d)
            nc.sync.dma_start(out=wq_t[:, 1:MD, :, :], in_=wq_v[:, 1:MD, :, :])
            nc.scalar.dma_start(out=wk_t[:, 1:MD, :, :], in_=wk_v[:, 1:MD, :, :])
            for k in range(0, CK, 2):
                nc.sync.dma_start(out=xt_t[:, k, HT:], in_=xT_v[:, k, HT:])
                nc.scalar.dma_start(
                    out=xt_t[:, k + 1, HT:], in_=xT_v[:, k + 1, HT:]
                )
            nc.sync.dma_start(out=wo_t[:, :, :], in_=wo_v)
            # ones column (index 64) for the l (softmax denominator) rows
            nc.vector.memset(v_t[:, :, :, D : D + 1], 1.0)
            nc.vector.memset(ones1_t[:, :], 1.0)

            # ---- filler infrastructure ----
            pool_names = ["qkvps", "fillps"]
            pool_idx = [0]

            def next_ps():
                name = pool_names[pool_idx[0] % 2]
                pool_idx[0] += 1
                return psum.tile([128, 512], F32, name=name, bufs=1)

            def qk_unit(w_t, dst_t, m, j):
                """q/k projection for (m, j): 8 accumulating MMs + CAST."""
                ps = next_ps()
                for k in range(CK):
                    yield lambda k=k, ps=ps: nc.tensor.matmul(
                        ps[:, :],
                        lhsT=w_t[:, m, k, :],
                        rhs=xt_t[:, k, 512 * j : 512 * (j + 1)],
                        start=(k == 0),
                        stop=(k == CK - 1),
                    )
                yield lambda ps=ps: nc.vector.tensor_copy(
                    dst_t[:, m, 512 * j : 512 * (j + 1)], ps[:, :]
                )

            def v_unit(ti):
                """v projection for t-tile ti: 8 accumulating MMs + CAST."""
                ps = next_ps()
                for k in range(CK):
                    yield lambda k=k, ps=ps: nc.tensor.matmul(
                        ps[:, :],
                        lhsT=xt_t[:, k, 128 * ti : 128 * (ti + 1)],
                        rhs=wv_t[:, k, :],
                        start=(k == 0),
                        stop=(k == CK - 1),
                    )
                yield lambda ps=ps: nc.vector.tensor_copy(
                    v_t[:, ti, :, 0:D],
                    ps[:, :].rearrange("p (h d) -> p h d", h=H_LOC),
                )

            out_tiles = {}

            def proj_unit(ti, ci):
                """proj half-tile (ti, ci): 4 accumulating MMs + COPY + DMA."""
                tt = slice(128 * ti, 128 * (ti + 1))
                cs = slice(512 * ci, 512 * (ci + 1))
                ot = out_tiles[ti]
                ps = next_ps()
                for hp2 in range(MD):
                    yield lambda hp2=hp2, ps=ps: nc.tensor.matmul(
                        ps[:, :],
                        lhsT=yt_t[:, hp2, tt],
                        rhs=wo_t[:, hp2, cs],
                        start=(hp2 == 0),
                        stop=(hp2 == MD - 1),
                    )

                def fin(ps=ps):
                    nc.vector.tensor_copy(ot[:, cs], ps[:, :])
                    nc.sync.dma_start(out=out_d[tt, cs], in_=ot[:, cs])

                yield fin

            def proj_chunk_units(j):
                for ti in range(4 * j, 4 * (j + 1)):
                    out_tiles[ti] = out_pool.tile([128, C], F32, name="ot")
                    yield from proj_unit(ti, 0)
                    yield from proj_unit(ti, 1)

            fillers = []

            def add_unit(gen):
                fillers.extend(gen)

            def drain_fillers(n):
                for _ in range(min(n, len(fillers))):
                    fillers.pop(0)()

            def drain_all_fillers():
                drain_fillers(len(fillers))

            # ---- startup wave: v tiles 0-5, k-major so each k-step
            # consumes x chunk k's first half as it streams in
            def wave0():
                spsT = psum.tile([128, 2, 512], F32, name="sps", bufs=2)
                accT = psum.tile([128, 2, 512], F32, name="acc", bufs=1)
                vps = [
                    psum.tile([128, 512], F32, name="qkvps", bufs=1),
                    psum.tile([128, 512], F32, name="fillps", bufs=1),
                    spsT[:, 0, :],
                    spsT[:, 1, :],
                    accT[:, 0, :],
                    accT[:, 1, :],
                ]
                tis = [0, 1, 2, 3, 4, 5]
                for k in range(CK):
                    for idx, ti in enumerate(tis):
                        nc.tensor.matmul(
                            vps[idx],
                            lhsT=xt_t[:, k, 128 * ti : 128 * (ti + 1)],
                            rhs=wv_t[:, k, :],
                            start=(k == 0),
                            stop=(k == CK - 1),
                        )
                for idx, ti in enumerate(tis):
                    nc.vector.tensor_copy(
                        v_t[:, ti, :, 0:D],
                        vps[idx].rearrange("p (h d) -> p h d", h=H_LOC),
                    )

            # ---- attention chunk with braided fillers ----
            def attn_j(hp, j, quota=0, pe_fin=False):
                tq0 = 512 * j
                nblk = 4 * j + 4  # causal: tk blocks 0 .. 4j+3
                acc = psum.tile([128, 2, 512], F32, name="acc", bufs=1)
                pend = []  # per-block AV pairs, flushed one pair behind
                emitted = [0]
                npair = nblk // 2

                def flush_pair():
                    for blk in pend[:2]:
                        for mm in blk:
                            nc.tensor.matmul(**mm)
                    del pend[:2]

                def emit_block(i):
                    tk = slice(128 * i, 128 * (i + 1))
                    diag = i - 4 * j
                    lo = 128 * diag if diag >= 0 else 0
                    tqs = slice(tq0 + lo, tq0 + 512)
                    sps = psum.tile([128, 2, 512], F32, name="sps", bufs=2)
                    for h2, lohi in ((0, slice(0, 64)), (1, slice(64, 128))):
                        nc.tensor.matmul(
                            sps[:, h2, lo:],
                            lhsT=kt_t[lohi, hp, tk],
                            rhs=qt_t[lohi, hp, tqs],
                            start=True,
                            stop=True,
                        )
                    if diag >= 0:  # block crosses the causal diagonal
                        dg = slice(lo, lo + 128)
                        nc.vector.tensor_add(
                            sps[:, :, dg], sps[:, :, dg], mask_t[:, :, :]
                        )
                    pt = pt_pool.tile([128, 2, 512], DT, name="pt")
                    nc.scalar.activation(
                        pt[:, :, lo:],
                        sps[:, :, lo:],
                        mybir.ActivationFunctionType.Exp,
                        scale=0.125,
                    )
                    pend.append(
                        [
                            dict(
                                out=acc[0 : D + 1, h2, lo:],
                                lhsT=v_t[:, i, 2 * hp + h2, :],
                                rhs=pt[:, h2, lo:],
                                start=(i == 0),
                                stop=(i == nblk - 1),
                            )
                            for h2 in range(2)
                        ]
                    )

                for p in range(npair):
                    want = min(quota, (quota * (p + 1)) // max(npair - 1, 1))
                    drain_fillers(want - emitted[0])
                    emitted[0] = want
                    emit_block(2 * p)
                    emit_block(2 * p + 1)
                    if len(pend) >= 4:
                        flush_pair()
                while pend:
                    flush_pair()

                # normalization: copy y/l rows to SBUF now (frees acc); the
                # rest is deferred.  Deferred chain (all off the PE): l row
                # -> partition 0 via SBUF->SBUF DMA, partition_broadcast on
                # GpSimd, reciprocal + multiplies on DVE.  pe_fin (last
                # chunk): replicate matmul for a shorter serial tail.
                tq = slice(tq0, tq0 + 512)
                if pe_fin:
                    yu = yu_pool.tile([D + 1, 2, 512], DT, name="yub", bufs=1)
                else:
                    yu = yu_pool.tile([D + 1, 2, 512], F32, name="yu")
                nc.vector.tensor_copy(yu[:, 0, :], acc[0 : D + 1, 0, :])
                nc.vector.tensor_copy(yu[:, 1, :], acc[0 : D + 1, 1, :])

                def finish_gpsimd():
                    lrow = lr_pool.tile([1, 2, 512], F32, name="lrow")
                    nc.sync.dma_start(out=lrow[:, :, :], in_=yu[D : D + 1, :, :])
                    lb = lb_pool.tile([64, 2, 512], F32, name="lb")
                    nc.gpsimd.partition_broadcast(lb[:, :, :], lrow[:, :, :])
                    rl = rl_pool.tile([64, 2, 512], F32, name="rl")
                    nc.vector.reciprocal_approx_fast(rl[:, :, :], lb[:, :, :])
                    nc.vector.tensor_mul(
                        yt_t[0:64, hp, tq], yu[0:D, 0, :], rl[:, 0, :]
                    )
                    nc.vector.tensor_mul(
                        yt_t[64:128, hp, tq], yu[0:D, 1, :], rl[:, 1, :]
                    )

                def finish_pe():
                    for h2 in range(2):
                        nc.tensor.matmul(
                            acc[0:64, h2, :],
                            lhsT=ones1_t[D : D + 1, :],
                            rhs=yu[D : D + 1, h2, :],
                            start=True,
                            stop=True,
                            tile_position=(64, 0),
                        )
                    rl = rl_pool.tile([64, 2, 512], F32, name="rl")
                    nc.vector.reciprocal_approx_fast(rl[:, 0, :], acc[0:64, 0, :])
                    nc.vector.reciprocal_approx_fast(rl[:, 1, :], acc[0:64, 1, :])
                    nc.vector.tensor_mul(
                        yt_t[0:64, hp, tq], yu[0:D, 0, :], rl[:, 0, :]
                    )
                    nc.vector.tensor_mul(
                        yt_t[64:128, hp, tq], yu[0:D, 1, :], rl[:, 1, :]
                    )

                return finish_pe if pe_fin else finish_gpsimd

            # ---- schedule ----
            pend_fin = [None]

            def run_fin():
                if pend_fin[0] is not None:
                    pend_fin[0]()
                    pend_fin[0] = None

            wave0()
            for step in qk_unit(wq_t, qt_t, 0, 0):
                step()
            for step in qk_unit(wk_t, kt_t, 0, 0):
                step()

            # stage m=0: braid v tiles and next qk chunk into attention
            for j in range(NJ):
                if j == 0:
                    add_unit(v_unit(6))
                    add_unit(v_unit(7))
                elif j < NJ - 1:
                    for ti in range(4 * (j + 1), 4 * (j + 2)):
                        add_unit(v_unit(ti))
                if j < NJ - 1:
                    add_unit(qk_unit(wq_t, qt_t, 0, j + 1))
                    add_unit(qk_unit(wk_t, kt_t, 0, j + 1))
                else:
                    add_unit(qk_unit(wq_t, qt_t, 1, 0))
                    add_unit(qk_unit(wk_t, kt_t, 1, 0))
                run_fin()
                quota = len(fillers)
                pend_fin[0] = attn_j(0, j, quota)
                drain_all_fillers()

            # stages m=1,2: braid the next chunk's qk
            for m in range(1, MD - 1):
                for j in range(NJ):
                    if j < NJ - 1:
                        add_unit(qk_unit(wq_t, qt_t, m, j + 1))
                        add_unit(qk_unit(wk_t, kt_t, m, j + 1))
                    else:
                        add_unit(qk_unit(wq_t, qt_t, m + 1, 0))
                        add_unit(qk_unit(wk_t, kt_t, m + 1, 0))
                    run_fin()
                    quota = len(fillers)
                    pend_fin[0] = attn_j(m, j, quota)
                    drain_all_fillers()

            # stage m=3: braid next qk + the previous chunk's output proj
            m = MD - 1
            for j in range(NJ):
                if j < NJ - 1:
                    add_unit(qk_unit(wq_t, qt_t, m, j + 1))
                    add_unit(qk_unit(wk_t, kt_t, m, j + 1))
                run_fin()  # finish of chunk j-1 -> yt(j-1) complete
                if j > 0:
                    add_unit(proj_chunk_units(j - 1))
                quota = len(fillers)
                pend_fin[0] = attn_j(m, j, quota, pe_fin=(j == NJ - 1))
                drain_all_fillers()
            # tail: finish last chunk (PE replicate variant), then its proj
            run_fin()
            for step in proj_chunk_units(NJ - 1):
                step()

    nc.compile()
    return nc


def make_host_inputs(x, w_qkv, w_proj, t_len=T):
    """Shard full inputs into the 8 per-core input dicts."""
    mask = np.where(
        np.arange(128)[None, :] >= np.arange(128)[:, None], 0.0, NEG
    ).astype(np.float32)

    def m_major(w):
        # [C, 512] -> [m, p, k, 128] with w[k*128+p, m*128+d]
        return np.ascontiguousarray(
            w.reshape(CK, 128, MD, 128).transpose(2, 1, 0, 3)
        )

    in_maps = []
    for c in range(8):
        b, g = c // 2, c % 2
        xT = np.ascontiguousarray(x[b][:t_len].T).astype(BF16)
        wq = m_major(w_qkv[:, 512 * g : 512 * (g + 1)].astype(BF16))
        wk = m_major(w_qkv[:, C + 512 * g : C + 512 * (g + 1)].astype(BF16))
        wv = w_qkv[:, 2 * C + 512 * g : 2 * C + 512 * (g + 1)].astype(BF16)
        wo = np.ascontiguousarray(w_proj[512 * g : 512 * (g + 1), :]).astype(BF16)
        in_maps.append(dict(xT=xT, wq=wq, wk=wk, wv=wv, wo=wo, mask=mask))
    return in_maps


_CACHE = {}


def _get_program():
    if "nc" not in _CACHE:
        _CACHE["nc"] = build_program()
    return _CACHE["nc"]


def kernel(x, w_qkv, w_proj, _trace=False, _trace_kwargs=None):
    x = np.asarray(x, np.float32)
    w_qkv = np.asarray(w_qkv, np.float32)
    w_proj = np.asarray(w_proj, np.float32)
    nc = _get_program()
    in_maps = make_host_inputs(x, w_qkv, w_proj)
    kw = {}
    if _trace:
        kw = dict(trace=True, **(_trace_kwargs or {}))
    res = run_bass_kernel_spmd(nc, in_maps, core_ids=list(range(8)), **kw)
    out = np.empty((B, T, C), np.float32)
    for b in range(B):
        out[b] = res.results[2 * b]["out"] + res.results[2 * b + 1]["out"]
    if _trace:
        return out, res
    return out


# revision 15
# speedup vs baseline: 1.0825x; 1.0089x over previous
"""Causal self-attention (B=4, T=2048, C=1024, H=16) on 8 trn2 NeuronCores.

Sharding: core c = (batch b = c//2, head-group g = c%2). Each core computes
the full attention for batch b and heads 8g..8g+7 (column-parallel qkv,
row-parallel proj), producing a partial [T, C] output; the host sums the two
partials per batch.

Per-core device kernel (Bass/Tile, SPMD same program on all 8 cores):
  qT/kT  [512, T] = (wq|wk).T @ x.T        (bf16 matmuls, fp32 psum)
  v      [T, 8, 65]  (natural layout, ones column appended per head)
  S^T    [tk 128, tq 512] blocks = kT.T-slices @ qT-slices (2 heads
         row-packed; the K=64 pair runs concurrently via PE row tiling)
  P^T    = exp((S^T + causal_mask)/8)      (ScalarE, psum->sbuf bf16)
  y/l    = [v|1].T @ P^T  accumulated over tk  -> [65, tq] psum per head
  yT_n   = yT * (1/l)   (l row moved to partition 0 by an SBUF->SBUF DMA,
         broadcast across partitions on GpSimd, recip+mul on DVE; the last
         chunk uses a PE replicate matmul for a shorter serial tail)
  out    = yT_n.T @ wo  -> [T, C] fp32 partial

Scheduling: the Tile scheduler is dependency+priority driven, so emission
order is a priority hint.  All non-attention matmuls (qkv projections, v
tiles, output proj) are braided INTO the attention block loop as "filler"
steps, front-loaded per chunk; filler units alternate between two
single-bank psum pools.  wq/wk are sent by the host in m-major layout so
each 128-column slice is a contiguous fast DMA; x is split by
(k-chunk, T-half) across the two HWDGE queues so the v/qk startup wave
starts as soon as wv + x chunk 0 land and attention by ~19us.
"""

import os
import sys

import numpy as np

import concourse.bacc as bacc
import concourse.bass as bass
import concourse.bass_utils as _bass_utils
import concourse.mybir as mybir
import concourse.tile as tile
from concourse import library_config
from concourse.bass_utils import run_bass_kernel_spmd

try:
    import ml_dtypes

    BF16 = np.dtype(ml_dtypes.bfloat16)
except ImportError:  # pragma: no cover
    BF16 = np.dtype("bfloat16")

# walrus's LDWEIGHTS optimization rejects tile_position'd ldweights
# ("InstLdweights is not compatible with LDW optimization"), so it stays off.
ENABLE_LDW_OPT = False
if ENABLE_LDW_OPT and not getattr(_bass_utils, "_ldw_patch", False):
    _orig_run_command = _bass_utils.run_command

    def _run_command_ldw(argv, **kwargs):
        argv = [
            "--enable-ldw-opt=true" if a == "--enable-ldw-opt=false" else a
            for a in argv
        ]
        return _orig_run_command(argv, **kwargs)

    _bass_utils.run_command = _run_command_ldw
    _bass_utils._ldw_patch = True

B, T, C = 4, 2048, 1024
N_HEAD = 16
D = 64  # head dim
H_LOC = 8  # heads per core
DL = H_LOC * D  # 512, local d width per core
CK = C // 128  # 8 contraction chunks
MD = DL // 128  # 4 d-chunks of qT/kT/yT
DT = mybir.dt.bfloat16
F32 = mybir.dt.float32
NEG = -1.0e9


def build_program(t_len=T, enable_asserts=False):
    """Build the SPMD per-core program. Returns the compiled Bacc object."""
    NJ = t_len // 512  # tq chunks
    NTT = t_len // 128  # 128-wide t tiles

    nc = bacc.Bacc(
        "TRN2",
        target_bir_lowering=False,
        debug=False,
        enable_asserts=enable_asserts,
        num_devices=8,
    )

    xT_d = nc.dram_tensor("xT", [C, t_len], DT, kind="ExternalInput").ap()
    # wq/wk arrive m-major: [m, p, k, 128] so each m-slice DMA is contiguous
    wq_d = nc.dram_tensor("wq", [MD, 128, CK, 128], DT, kind="ExternalInput").ap()
    wk_d = nc.dram_tensor("wk", [MD, 128, CK, 128], DT, kind="ExternalInput").ap()
    wv_d = nc.dram_tensor("wv", [C, DL], DT, kind="ExternalInput").ap()
    wo_d = nc.dram_tensor("wo", [DL, C], DT, kind="ExternalInput").ap()
    mask_d = nc.dram_tensor("mask", [128, 128], F32, kind="ExternalInput").ap()
    out_d = nc.dram_tensor("out", [t_len, C], F32, kind="ExternalOutput").ap()

    with tile.TileContext(nc) as tc:
        with (
            tc.tile_pool(name="consts", bufs=1) as cpool,
            tc.tile_pool(name="ptp", bufs=6) as pt_pool,
            tc.tile_pool(name="yup", bufs=3) as yu_pool,
            tc.tile_pool(name="lrp", bufs=3) as lr_pool,
            tc.tile_pool(name="lbp", bufs=3) as lb_pool,
            tc.tile_pool(name="rlp", bufs=3) as rl_pool,
            tc.tile_pool(name="outp", bufs=3) as out_pool,
            tc.tile_pool(name="psum", bufs=1, space="PSUM") as psum,
        ):
            # ---- persistent SBUF tensors ----
            xt_t = cpool.tile([128, CK, t_len], DT, name="xt")
            wq_t = cpool.tile([128, MD, CK, 128], DT, name="wqt")
            wk_t = cpool.tile([128, MD, CK, 128], DT, name="wkt")
            wv_t = cpool.tile([128, CK, DL], DT, name="wvt")
            wo_t = cpool.tile([128, MD, C], DT, name="wot")
            qt_t = cpool.tile([128, MD, t_len], DT, name="qtt")
            kt_t = cpool.tile([128, MD, t_len], DT, name="ktt")
            v_t = cpool.tile([128, NTT, H_LOC, D + 1], DT, name="vt")
            yt_t = cpool.tile([128, MD, t_len], DT, name="ytt")
            mask_t = cpool.tile([128, 2, 128], F32, name="maskt")
            ones1_t = cpool.tile([D + 1, 64], DT, name="ones1t")

            # gpsimd library for partition_broadcast (normalization)
            nc.gpsimd.load_library(library_config.attn)

            # ---- input DMAs: x split by (k-chunk, T-half) alternating the
            # two HWDGE queues; wq/wk m-slices are contiguous (host m-major)
            xT_v = xT_d.rearrange("(k p) t -> p k t", p=128)
            wq_v = wq_d.rearrange("m p k d -> p m k d")
            wk_v = wk_d.rearrange("m p k d -> p m k d")
            wv_v = wv_d.rearrange("(k p) d -> p k d", p=128)
            wo_v = wo_d.rearrange("(m p) c -> p m c", p=128)
            HT = t_len // 2
            nc.sync.dma_start(out=wv_t[:, 0:4, :], in_=wv_v[:, 0:4, :])
            nc.scalar.dma_start(out=wv_t[:, 4:CK, :], in_=wv_v[:, 4:CK, :])
            for k in range(0, CK, 2):
                nc.sync.dma_start(out=xt_t[:, k, 0:HT], in_=xT_v[:, k, 0:HT])
                nc.scalar.dma_start(
                    out=xt_t[:, k + 1, 0:HT], in_=xT_v[:, k + 1, 0:HT]
                )
                if k == 0:
                    nc.sync.dma_start(out=wq_t[:, 0, :, :], in_=wq_v[:, 0, :, :])
                    nc.scalar.dma_start(
                        out=wk_t[:, 0, :, :], in_=wk_v[:, 0, :, :]
                    )
                    nc.scalar.dma_start(out=mask_t[:, 0, :], in_=mask_d)
                    nc.scalar.dma_start(out=mask_t[:, 1, :], in_=mask_d)
            nc.sync.dma_start(out=wq_t[:, 1:MD, :, :], in_=wq_v[:, 1:MD, :, :])
            nc.scalar.dma_start(out=wk_t[:, 1:MD, :, :], in_=wk_v[:, 1:MD, :, :])
            for k in range(0, CK, 2):
                nc.sync.dma_start(out=xt_t[:, k, HT:], in_=xT_v[:, k, HT:])
                nc.scalar.dma_start(
                    out=xt_t[:, k + 1, HT:], in_=xT_v[:, k + 1, HT:]
                )
            nc.sync.dma_start(out=wo_t[:, :, :], in_=wo_v)
            # ones column (index 64) for the l (softmax denominator) rows
            nc.vector.memset(v_t[:, :, :, D : D + 1], 1.0)
            nc.vector.memset(ones1_t[:, :], 1.0)

            # ---- filler infrastructure ----
            pool_names = ["qkvps", "fillps"]
            pool_idx = [0]

            def next_ps():
                name = pool_names[pool_idx[0] % 2]
                pool_idx[0] += 1
                return psum.tile([128, 512], F32, name=name, bufs=1)

            def qk_unit(w_t, dst_t, m, j):
                """q/k projection for (m, j): 8 accumulating MMs + CAST."""
                ps = next_ps()
                for k in range(CK):
                    yield lambda k=k, ps=ps: nc.tensor.matmul(
                        ps[:, :],
                        lhsT=w_t[:, m, k, :],
                        rhs=xt_t[:, k, 512 * j : 512 * (j + 1)],
                        start=(k == 0),
                        stop=(k == CK - 1),
                    )
                yield lambda ps=ps: nc.vector.tensor_copy(
                    dst_t[:, m, 512 * j : 512 * (j + 1)], ps[:, :]
                )

            def v_unit(ti):
                """v projection for t-tile ti: 8 accumulating MMs + CAST."""
                ps = next_ps()
                for k in range(CK):
                    yield lambda k=k, ps=ps: nc.tensor.matmul(
                        ps[:, :],
                        lhsT=xt_t[:, k, 128 * ti : 128 * (ti + 1)],
                        rhs=wv_t[:, k, :],
                        start=(k == 0),
                        stop=(k == CK - 1),
                    )
                yield lambda ps=ps: nc.vector.tensor_copy(
                    v_t[:, ti, :, 0:D],
                    ps[:, :].rearrange("p (h d) -> p h d", h=H_LOC),
                )

            out_tiles = {}

            def proj_unit(ti, ci):
                """proj half-tile (ti, ci): 4 accumulating MMs + COPY + DMA."""
                tt = slice(128 * ti, 128 * (ti + 1))
                cs = slice(512 * ci, 512 * (ci + 1))
                ot = out_tiles[ti]
                ps = next_ps()
                for hp2 in range(MD):
                    yield lambda hp2=hp2, ps=ps: nc.tensor.matmul(
                        ps[:, :],
                        lhsT=yt_t[:, hp2, tt],
                        rhs=wo_t[:, hp2, cs],
                        start=(hp2 == 0),
                        stop=(hp2 == MD - 1),
                    )

                def fin(ps=ps):
                    nc.vector.tensor_copy(ot[:, cs], ps[:, :])
                    nc.sync.dma_start(out=out_d[tt, cs], in_=ot[:, cs])

                yield fin

            def proj_chunk_units(j):
                for ti in range(4 * j, 4 * (j + 1)):
                    out_tiles[ti] = out_pool.tile([128, C], F32, name="ot")
                    yield from proj_unit(ti, 0)
                    yield from proj_unit(ti, 1)

            fillers = []

            def add_unit(gen):
                fillers.extend(gen)

            def drain_fillers(n):
                for _ in range(min(n, len(fillers))):
                    fillers.pop(0)()

            def drain_all_fillers():
                drain_fillers(len(fillers))

            # ---- startup wave: v tiles 0-5, k-major so each k-step
            # consumes x chunk k's first half as it streams in
            def wave0():
                spsT = psum.tile([128, 2, 512], F32, name="sps", bufs=2)
                accT = psum.tile([128, 2, 512], F32, name="acc", bufs=1)
                vps = [
                    psum.tile([128, 512], F32, name="qkvps", bufs=1),
                    psum.tile([128, 512], F32, name="fillps", bufs=1),
                    spsT[:, 0, :],
                    spsT[:, 1, :],
                    accT[:, 0, :],
                    accT[:, 1, :],
                ]
                tis = [0, 1, 2, 3, 4, 5]
                for k in range(CK):
                    for idx, ti in enumerate(tis):
                        nc.tensor.matmul(
                            vps[idx],
                            lhsT=xt_t[:, k, 128 * ti : 128 * (ti + 1)],
                            rhs=wv_t[:, k, :],
                            start=(k == 0),
                            stop=(k == CK - 1),
                        )
                for idx, ti in enumerate(tis):
                    nc.vector.tensor_copy(
                        v_t[:, ti, :, 0:D],
                        vps[idx].rearrange("p (h d) -> p h d", h=H_LOC),
                    )

            # ---- attention chunk with braided fillers ----
            def attn_j(hp, j, quota=0, pe_fin=False):
                tq0 = 512 * j
                nblk = 4 * j + 4  # causal: tk blocks 0 .. 4j+3
                acc = psum.tile([128, 2, 512], F32, name="acc", bufs=1)
                pend = []  # per-block AV pairs, flushed one pair behind
                emitted = [0]
                npair = nblk // 2

                def flush_pair():
                    for blk in pend[:2]:
                        for mm in blk:
                            nc.tensor.matmul(**mm)
                    del pend[:2]

                def emit_block(i):
                    tk = slice(128 * i, 128 * (i + 1))
                    diag = i - 4 * j
                    lo = 128 * diag if diag >= 0 else 0
                    tqs = slice(tq0 + lo, tq0 + 512)
                    sps = psum.tile([128, 2, 512], F32, name="sps", bufs=2)
                    for h2, lohi in ((0, slice(0, 64)), (1, slice(64, 128))):
                        nc.tensor.matmul(
                            sps[:, h2, lo:],
                            lhsT=kt_t[lohi, hp, tk],
                            rhs=qt_t[lohi, hp, tqs],
                            start=True,
                            stop=True,
                        )
                    if diag >= 0:  # block crosses the causal diagonal
                        dg = slice(lo, lo + 128)
                        nc.vector.tensor_add(
                            sps[:, :, dg], sps[:, :, dg], mask_t[:, :, :]
                        )
                    pt = pt_pool.tile([128, 2, 512], DT, name="pt")
                    nc.scalar.activation(
                        pt[:, :, lo:],
                        sps[:, :, lo:],
                        mybir.ActivationFunctionType.Exp,
                        scale=0.125,
                    )
                    pend.append(
                        [
                            dict(
                                out=acc[0 : D + 1, h2, lo:],
                                lhsT=v_t[:, i, 2 * hp + h2, :],
                                rhs=pt[:, h2, lo:],
                                start=(i == 0),
                                stop=(i == nblk - 1),
                            )
                            for h2 in range(2)
                        ]
                    )

                for p in range(npair):
                    want = min(quota, (quota * (p + 1)) // max(npair - 1, 1))
                    drain_fillers(want - emitted[0])
                    emitted[0] = want
                    emit_block(2 * p)
                    emit_block(2 * p + 1)
                    if len(pend) >= 4:
                        flush_pair()
                while pend:
                    flush_pair()

                # normalization: copy y/l rows to SBUF now (frees acc); the
                # rest is deferred.  Deferred chain (all off the PE): l row
                # -> partition 0 via SBUF->SBUF DMA, partition_broadcast on
                # GpSimd, reciprocal + multiplies on DVE.  pe_fin (last
                # chunk): replicate matmul for a shorter serial tail.
                tq = slice(tq0, tq0 + 512)
                if pe_fin:
                    # last chunk: split the two copies across ScalarE/DVE so
                    # the serial tail chain is shorter
                    yu = yu_pool.tile([D + 1, 2, 512], DT, name="yub", bufs=1)
                    nc.scalar.activation(
                        yu[:, 0, :],
                        acc[0 : D + 1, 0, :],
                        mybir.ActivationFunctionType.Copy,
                    )
                    nc.vector.tensor_copy(yu[:, 1, :], acc[0 : D + 1, 1, :])
                else:
                    yu = yu_pool.tile([D + 1, 2, 512], F32, name="yu")
                    nc.vector.tensor_copy(yu[:, 0, :], acc[0 : D + 1, 0, :])
                    nc.vector.tensor_copy(yu[:, 1, :], acc[0 : D + 1, 1, :])

                def finish_gpsimd():
                    lrow = lr_pool.tile([1, 2, 512], F32, name="lrow")
                    nc.sync.dma_start(out=lrow[:, :, :], in_=yu[D : D + 1, :, :])
                    lb = lb_pool.tile([64, 2, 512], F32, name="lb")
                    nc.gpsimd.partition_broadcast(lb[:, :, :], lrow[:, :, :])
                    rl = rl_pool.tile([64, 2, 512], F32, name="rl")
                    nc.vector.reciprocal_approx_fast(rl[:, :, :], lb[:, :, :])
                    nc.vector.tensor_mul(
                        yt_t[0:64, hp, tq], yu[0:D, 0, :], rl[:, 0, :]
                    )
                    nc.vector.tensor_mul(
                        yt_t[64:128, hp, tq], yu[0:D, 1, :], rl[:, 1, :]
                    )

                def finish_pe():
                    for h2 in range(2):
                        nc.tensor.matmul(
                            acc[0:64, h2, :],
                            lhsT=ones1_t[D : D + 1, :],
                            rhs=yu[D : D + 1, h2, :],
                            start=True,
                            stop=True,
                            tile_position=(64, 0),
                        )
                    rl = rl_pool.tile([64, 2, 512], F32, name="rl")
                    nc.vector.reciprocal_approx_fast(rl[:, 0, :], acc[0:64, 0, :])
                    nc.vector.reciprocal_approx_fast(rl[:, 1, :], acc[0:64, 1, :])
                    nc.vector.tensor_mul(
                        yt_t[0:64, hp, tq], yu[0:D, 0, :], rl[:, 0, :]
                    )
                    nc.vector.tensor_mul(
                        yt_t[64:128, hp, tq], yu[0:D, 1, :], rl[:, 1, :]
                    )

                return finish_pe if pe_fin else finish_gpsimd

            # ---- schedule ----
            pend_fin = [None]

            def run_fin():
                if pend_fin[0] is not None:
                    pend_fin[0]()
                    pend_fin[0] = None

            wave0()
            for step in qk_unit(wq_t, qt_t, 0, 0):
                step()
            for step in qk_unit(wk_t, kt_t, 0, 0):
                step()

            # stage m=0: braid v tiles and next qk chunk into attention
            for j in range(NJ):
                if j == 0:
                    add_unit(v_unit(6))
                    add_unit(v_unit(7))
                elif j < NJ - 1:
                    for ti in range(4 * (j + 1), 4 * (j + 2)):
                        add_unit(v_unit(ti))
                if j < NJ - 1:
                    add_unit(qk_unit(wq_t, qt_t, 0, j + 1))
                    add_unit(qk_unit(wk_t, kt_t, 0, j + 1))
                else:
                    add_unit(qk_unit(wq_t, qt_t, 1, 0))
                    add_unit(qk_unit(wk_t, kt_t, 1, 0))
                run_fin()
                quota = len(fillers)
                pend_fin[0] = attn_j(0, j, quota)
                drain_all_fillers()

            # stages m=1,2: braid the next chunk's qk
            for m in range(1, MD - 1):
                for j in range(NJ):
                    if j < NJ - 1:
                        add_unit(qk_unit(wq_t, qt_t, m, j + 1))
                        add_unit(qk_unit(wk_t, kt_t, m, j + 1))
                    else:
                        add_unit(qk_unit(wq_t, qt_t, m + 1, 0))
                        add_unit(qk_unit(wk_t, kt_t, m + 1, 0))
                    run_fin()
                    quota = len(fillers)
                    pend_fin[0] = attn_j(m, j, quota)
                    drain_all_fillers()

            # stage m=3: braid next qk + the previous chunk's output proj
            m = MD - 1
            reserve = []
            for j in range(NJ):
                if j < NJ - 1:
                    add_unit(qk_unit(wq_t, qt_t, m, j + 1))
                    add_unit(qk_unit(wk_t, kt_t, m, j + 1))
                run_fin()  # finish of chunk j-1 -> yt(j-1) complete
                if j > 0:
                    steps = list(proj_chunk_units(j - 1))
                    if j == NJ - 1:
                        # hold back half of proj(j-1) as PE work to keep the
                        # PE warm through the final fin chain
                        fillers.extend(steps[: len(steps) // 2])
                        reserve = steps[len(steps) // 2 :]
                    else:
                        fillers.extend(steps)
                quota = len(fillers)
                pend_fin[0] = attn_j(m, j, quota, pe_fin=(j == NJ - 1))
                drain_all_fillers()
            # tail: the reserved proj(NJ-2) halves run during the final fin
            # chain, then the last chunk's proj
            for step in reserve:
                step()
            run_fin()
            for step in proj_chunk_units(NJ - 1):
                step()

    nc.compile()
    return nc


def make_host_inputs(x, w_qkv, w_proj, t_len=T):
    """Shard full inputs into the 8 per-core input dicts."""
    mask = np.where(
        np.arange(128)[None, :] >= np.arange(128)[:, None], 0.0, NEG
    ).astype(np.float32)

    def m_major(w):
        # [C, 512] -> [m, p, k, 128] with w[k*128+p, m*128+d]
        return np.ascontiguousarray(
            w.reshape(CK, 128, MD, 128).transpose(2, 1, 0, 3)
        )

    in_maps = []
    for c in range(8):
        b, g = c // 2, c % 2
        xT = np.ascontiguousarray(x[b][:t_len].T).astype(BF16)
        wq = m_major(w_qkv[:, 512 * g : 512 * (g + 1)].astype(BF16))
        wk = m_major(w_qkv[:, C + 512 * g : C + 512 * (g + 1)].astype(BF16))
        wv = w_qkv[:, 2 * C + 512 * g : 2 * C + 512 * (g + 1)].astype(BF16)
        wo = np.ascontiguousarray(w_proj[512 * g : 512 * (g + 1), :]).astype(BF16)
        in_maps.append(dict(xT=xT, wq=wq, wk=wk, wv=wv, wo=wo, mask=mask))
    return in_maps


_CACHE = {}


def _get_program():
    if "nc" not in _CACHE:
        _CACHE["nc"] = build_program()
    return _CACHE["nc"]


def kernel(x, w_qkv, w_proj, _trace=False, _trace_kwargs=None):
    x = np.asarray(x, np.float32)
    w_qkv = np.asarray(w_qkv, np.float32)
    w_proj = np.asarray(w_proj, np.float32)
    nc = _get_program()
    in_maps = make_host_inputs(x, w_qkv, w_proj)
    kw = {}
    if _trace:
        kw = dict(trace=True, **(_trace_kwargs or {}))
    res = run_bass_kernel_spmd(nc, in_maps, core_ids=list(range(8)), **kw)
    out = np.empty((B, T, C), np.float32)
    for b in range(B):
        out[b] = res.results[2 * b]["out"] + res.results[2 * b + 1]["out"]
    if _trace:
        return out, res
    return out
